# revision 23
# baseline (speedup 1.0000x reference)
"""BitConv2dInfer on 8 Trainium2 NeuronCores — fp8 DoubleRow version.

Reference computation (per full input):
    x = clip(x, -1, 1)                       # x [32, 256, 56, 56] f32
    y = conv2d(x, w_q, pad=1)                # w_q [256, 256, 3, 3] ternary
    y = y * s + bias                         # per-out-channel affine
Sharding: data-parallel over batch — each of the 8 cores gets 4 images and
the full (tiny) weights; outputs concatenate over batch with no comms.

Key ideas vs the bf16 version:
  - Activations are clipped to [-1,1] and weights are ternary, so fp8 e4m3
    holds the weights exactly and quantizes x with max elem err 2^-5; the
    resulting output rel-err is ~1.6e-2 (measured against the reference on
    the real inputs), inside the 2e-2 gate.
  - fp8 enables MatmulPerfMode.DoubleRow: one matmul per conv tap contracts
    both 128-deep cin tiles at once (lhsT [128, 2, 128], rhs [128, 2, N]) at
    2x the bf16 FLOP rate (~157 TF/s measured: ~454 PE cycles per 448-col
    group-tap at 2.4 GHz).
  - x is clipped, quantized, padded (1-px border) and laid out
    partition-major on the HOST, so the device does no clamp at all and
    input DMAs are fully contiguous per partition.
  - All x transfers share the scalar HWDGE FIFO (image 0's first rows
    lead) so they drain in priority order without stealing HBM bandwidth
    from each other; weights ride the sync FIFO.
  - rhs windows are strided 4D APs [128, ci, rows, 56] straight into the
    padded tile, so the moving stream carries no junk pad columns.
  - ACT engine evacuates PSUM (dense reads) with per-partition scale+bias
    fused; outputs stream back on the sync HWDGE queue, with the final
    output tile tapered so the closing ACT+DMA drain fast.

The PE clock gate (HAM) starts at 1.2 GHz and only reaches 2.4 GHz after
~3.2us of sustained activity, so the kernel front-runs dummy matmuls on a
zeroed tile while the first input chunks are in flight. The warm matmuls
must be full-size: a trial with many short warm matmuls left the PE stuck
~20% below peak clock for the entire run (135us vs 115us).
"""

import sys

sys.path.insert(0, "/opt/trn_rl_repo")

import ml_dtypes
import numpy as np

import concourse.bass as bass  # noqa: F401  (registers engines)
import concourse.mybir as mybir
import concourse.tile as tile
from concourse import bacc
from concourse.bass_utils import run_bass_kernel_spmd

N, CIN, COUT, H, W = 32, 256, 256, 56, 56
NCORES = 8
NB = N // NCORES          # images per core
HP, WP = H + 2, W + 2     # padded spatial
RG = 8                    # output rows per PSUM group (8*56=448 <= 512 f32/bank)
NCH = H // RG             # full groups per image
NCI = CIN // 128          # cin tiles
NCO = COUT // 128         # cout tiles
NTAP = 9
R0 = 27                   # image-0 second chunk end (chunks: 0:11, 11:27, 27:58)
N_WARM_MM = 6             # dummy matmuls to lift the HAM clock gate

_compiled = {}


def _build():
    nc = bacc.Bacc("TRN2", target_bir_lowering=False, debug=False)
    f32, fp8 = mybir.dt.float32, mybir.dt.float8e4
    x_d = nc.dram_tensor("x", [NB, 128, NCI, HP, WP], fp8, kind="ExternalInput").ap()
    w_d = nc.dram_tensor(
        "w", [128, NCO, NTAP, NCI, 128], fp8, kind="ExternalInput"
    ).ap()
    sb_d = nc.dram_tensor("sb", [128, 2 * NCO], f32, kind="ExternalInput").ap()
    o_d = nc.dram_tensor("out", [NB, COUT, H, W], f32, kind="ExternalOutput").ap()

    dr = mybir.MatmulPerfMode.DoubleRow

    with tile.TileContext(nc) as tc:
        with (
            tc.tile_pool(name="const", bufs=1) as cpool,
            tc.tile_pool(name="osb", bufs=4) as opool,
            tc.tile_pool(name="ps", bufs=7, space="PSUM") as pspool,
            tc.tile_pool(name="warmps", bufs=1, space="PSUM") as wpspool,
        ):
            w_sb = cpool.tile([128, NCO, NTAP, NCI, 128], fp8, tag="w")
            sb_sb = cpool.tile([128, 2 * NCO], f32, tag="sb")
            x_sb = [
                cpool.tile([128, NCI, HP, WP], fp8, tag=f"x{n}", name=f"x{n}")
                for n in range(NB)
            ]

            # Warm tile zeroed on gpsimd (its preamble retires earliest) so
            # the PE can start burning the HAM ramp ASAP.
            warm = cpool.tile([128, NCI, RG * W], fp8, tag="warm")
            nc.gpsimd.memset(warm[:], 0.0)

            # Critical path: w cout0 on sync, image-0 lead chunk on scalar.
            # All x transfers share the scalar HWDGE FIFO so they drain in
            # priority order without stealing HBM bandwidth from each other.
            nc.sync.dma_start(out=w_sb[:, 0], in_=w_d[:, 0])
            nc.scalar.dma_start(out=x_sb[0][:, :, 0:10], in_=x_d[0][:, :, 0:10])
            nc.scalar.dma_start(out=x_sb[0][:, :, 10:R0], in_=x_d[0][:, :, 10:R0])
            nc.scalar.dma_start(out=x_sb[0][:, :, R0:HP], in_=x_d[0][:, :, R0:HP])
            nc.scalar.dma_start(out=x_sb[1][:], in_=x_d[1])
            nc.sync.dma_start(out=w_sb[:, 1], in_=w_d[:, 1])
            nc.sync.dma_start(out=sb_sb[:], in_=sb_d)

            warm_ps = wpspool.tile([128, RG * W], f32, tag="warmps")
            for _ in range(N_WARM_MM):
                nc.tensor.matmul(
                    out=warm_ps[:], lhsT=warm[:, :, 0:128], rhs=warm[:],
                    start=True, stop=True, perf_mode=dr,
                )

            for n in range(NB):
                for co in range(NCO):
                    last_tile = n == NB - 1 and co == NCO - 1
                    osb = opool.tile([128, H, W], f32, tag="osb")
                    if last_tile:
                        # Taper the last groups so the closing ACT + DMA are
                        # small and the tail drains fast.
                        groups = [(c * RG, RG) for c in range(NCH - 1)]
                        groups += [(H - 8, 4), (H - 4, 2), (H - 2, 1), (H - 1, 1)]
                    else:
                        groups = [(c * RG, RG) for c in range(NCH)]
                    for g0, gn in groups:
                        ps = pspool.tile([128, RG, W], f32, tag="ps")
                        for t in range(NTAP):
                            kh, kw = divmod(t, 3)
                            nc.tensor.matmul(
                                out=ps[:, 0:gn, :],
                                lhsT=w_sb[:, co, t],
                                rhs=x_sb[n][:, :, g0 + kh:g0 + kh + gn, kw:kw + W],
                                start=(t == 0),
                                stop=(t == NTAP - 1),
                                perf_mode=dr,
                            )
                        nc.scalar.activation(
                            out=osb[:, g0:g0 + gn, :], in_=ps[:, 0:gn, :],
                            func=mybir.ActivationFunctionType.Identity,
                            bias=sb_sb[:, NCO + co:NCO + co + 1],
                            scale=sb_sb[:, co:co + 1],
                        )
                        if last_tile:
                            nc.sync.dma_start(
                                out=o_d[n, co * 128:(co + 1) * 128, g0:g0 + gn],
                                in_=osb[:, g0:g0 + gn],
                            )
                    if n == 0 and co == 0:
                        # Remaining input images: issued here so their queue
                        # slots sit behind image 0/1's transfers and the
                        # first ACTs, but still land far ahead of use.
                        nc.scalar.dma_start(out=x_sb[2][:], in_=x_d[2])
                        nc.scalar.dma_start(out=x_sb[3][:], in_=x_d[3])
                    dst = o_d[n, co * 128:(co + 1) * 128]
                    if not last_tile:
                        nc.sync.dma_start(out=dst[:, 0:32], in_=osb[:, 0:32])
                        nc.sync.dma_start(out=dst[:, 32:H], in_=osb[:, 32:H])

    nc.compile()
    return nc


def _prep_inputs(x, w_q, s, bias):
    fp8 = ml_dtypes.float8_e4m3
    # x: clip, quantize, pad borders, partition-major [N, 128, NCI, HP, WP].
    xq = np.clip(x, -1.0, 1.0).astype(fp8)          # [N, CIN, H, W]
    xp = np.zeros((N, 128, NCI, HP, WP), dtype=fp8)
    xp[:, :, :, 1:H + 1, 1:W + 1] = xq.reshape(N, NCI, 128, H, W).transpose(
        0, 2, 1, 3, 4
    )
    # w: lhsT DoubleRow layout [k, co, tap, ci, j] so that
    # w_t[k, co, t, ci, j] = w_q[co*128 + j, ci*128 + k, kh, kw]
    w_t = (
        w_q.astype(np.float32)
        .reshape(NCO, 128, NCI, 128, NTAP)              # [co, j, ci, k, kh*kw]
        .transpose(3, 0, 4, 2, 1)                       # [k, co, tap, ci, j]
        .astype(fp8)
    )
    sb_t = np.concatenate(
        [
            np.ascontiguousarray(s.reshape(NCO, 128).T.astype(np.float32)),
            np.ascontiguousarray(bias.reshape(NCO, 128).T.astype(np.float32)),
        ],
        axis=1,
    )
    return xp, w_t, np.ascontiguousarray(sb_t)


def kernel(x, w_q, s, bias):
    if "nc" not in _compiled:
        _compiled["nc"] = _build()
    nc = _compiled["nc"]

    xp, w_t, sb_t = _prep_inputs(
        np.asarray(x, dtype=np.float32), np.asarray(w_q), np.asarray(s),
        np.asarray(bias),
    )
    core_ids = list(range(NCORES))
    in_maps = [
        {"x": xp[i * NB:(i + 1) * NB], "w": w_t, "sb": sb_t}
        for i in core_ids
    ]
    res = run_bass_kernel_spmd(nc, in_maps, core_ids)
    return np.concatenate([res.results[i]["out"] for i in core_ids], axis=0)
